# revision 25
# baseline (speedup 1.0000x reference)
"""Causal self-attention TRN2 Bass kernel.

Problem: B=4, T=2048, C=1024, H=16 heads (HD=64), torch-Linear semantics
(y = x @ W.T + b), causal + padding mask, softmax, output projection.

Sharding: 8 cores = (batch b in 0..3) x (head-half in 0..1). Each core
handles one batch and 8 heads (512 of the 1024 channels of QKV / of the
contraction dim of the output projection). The two half-cores of a batch
produce partial output projections that the host sums (plus bp).

Per-core kernel (all matmuls in float32r — full PE rate, ~1.5e-4 rel):
  Phase 1: QKV projections.
    Q^T, K^T computed head-major ([outch, T]) so attention needs no
    transposes; V computed token-major ([T, outch]) with an interleaved
    ones column per head (rowsum trick). Attention scale (1/8) and bias
    are folded in during the PSUM->SBUF copy on DVE; V bias is added via
    a K=1 ones-outer-product matmul into PSUM.
  Phase 2: flash-style causal attention per head-pair g (2 heads packed
    on PE rows 0-63 / 64-127 via tile_position for the K=64 S^T matmuls).
    S^T[k,q] = K Q^T tiles; causal masking adds -1e30 to PSUM on the
    diagonal tiles; exp on ACT (no max subtraction needed: |S|<~3);
    O_unnorm^T[d,q] plus rowsum row via [V | 1] stationary; normalization
    via reciprocal + K=1 broadcast matmul + DVE multiply into Y^T.
  Phase 3: output projection from Y^T tiles (stationary) vs Wp^T slices.
"""

import ml_dtypes
import numpy as np

import concourse.mybir as mybir
import concourse.tile as tile
from concourse import bacc
from concourse.bass_utils import run_bass_kernel_spmd

F32 = mybir.dt.float32
F32R = mybir.dt.float32r
BF16 = mybir.dt.bfloat16
AF = mybir.ActivationFunctionType
ALU = mybir.AluOpType

B, T, C, H = 4, 2048, 1024, 16
HD = C // H          # 64
IC = C // 2          # 512 channels per core (8 heads)
NKT = T // 128       # 16 k-tiles
NQC = T // 512       # 4 q-chunks
NCT = C // 128       # 8 contraction tiles for QKV
NEG = -1.0e30
SCALE = 1.0 / np.sqrt(HD)

_CACHE = {}


def _build():
    nc = bacc.Bacc("TRN2", target_bir_lowering=False, debug=False)

    xT_d = nc.dram_tensor("xT", [C, T], F32, kind="ExternalInput").ap()
    WqT_d = nc.dram_tensor("WqT", [C, IC], F32, kind="ExternalInput").ap()
    WkT_d = nc.dram_tensor("WkT", [C, IC], F32, kind="ExternalInput").ap()
    WvT_d = nc.dram_tensor("WvT", [C, IC], F32, kind="ExternalInput").ap()
    WpT_d = nc.dram_tensor("WpT", [IC, C], F32, kind="ExternalInput").ap()
    bq_d = nc.dram_tensor("bqs", [128, 4], F32, kind="ExternalInput").ap()
    bk_d = nc.dram_tensor("bks", [128, 4], F32, kind="ExternalInput").ap()
    bv_d = nc.dram_tensor("bvr", [1, IC], F32, kind="ExternalInput").ap()
    pad_d = nc.dram_tensor("padb", [128, NKT], F32, kind="ExternalInput").ap()
    mask_d = nc.dram_tensor("maskneg", [128, 896], F32, kind="ExternalInput").ap()
    ones128_d = nc.dram_tensor("ones128", [1, 128], F32, kind="ExternalInput").ap()
    ones8f_d = nc.dram_tensor("ones8", [128, 8], F32, kind="ExternalInput").ap()
    ident_d = nc.dram_tensor("ident", [128, 128], BF16, kind="ExternalInput").ap()
    maskb_d = nc.dram_tensor("maskb", [128, 128], BF16, kind="ExternalInput").ap()
    out_d = nc.dram_tensor("out", [T, C], F32, kind="ExternalOutput").ap()

    with tile.TileContext(nc) as tc:
        with tc.tile_pool(name="pp", bufs=1) as pp:
            # Persistent SBUF state
            QT = pp.tile([128, 4 * T], BF16, name="QT")     # 4 head-pair tiles
            KT = pp.tile([128, 4 * T], BF16, name="KT")
            Vt = pp.tile([128, NKT * 520], F32R, name="Vt")  # [V|1] x 8 heads
            YT = pp.tile([128, 4 * T], F32R, name="YT")
            bq_sb = pp.tile([128, 4], F32, name="bq_sb")
            bk_sb = pp.tile([128, 4], F32, name="bk_sb")
            bv_sb = pp.tile([1, IC], F32R, name="bv_sb")
            pad_sb = pp.tile([128, NKT], F32, name="pad_sb")
            ones128 = pp.tile([1, 128], F32R, name="ones128")
            maskf_sb = pp.tile([128, 128], F32, name="maskf_sb")
            nc.sync.dma_start(out=bq_sb[:], in_=bq_d)
            nc.sync.dma_start(out=bk_sb[:], in_=bk_d)
            nc.sync.dma_start(out=bv_sb[:], in_=bv_d.bitcast(F32R))
            nc.sync.dma_start(out=pad_sb[:], in_=pad_d)
            nc.sync.dma_start(out=ones128[:], in_=ones128_d.bitcast(F32R))
            nc.sync.dma_start(out=maskf_sb[:], in_=mask_d[:, 384:512])
            Vr = Vt.rearrange("p (k h c) -> p k h c", k=NKT, h=8, c=65)
            for kt in range(NKT):
                nc.sync.dma_start(out=Vr[:, kt, :, 64], in_=ones8f_d.bitcast(F32R))

            # ---- interleaved QKV production + causal attention ----
            # For each T-chunk ch: project Q^T/K^T/V for those 512 tokens,
            # then immediately run attention q-chunk qc=ch (its K range only
            # needs chunks <= ch). ACT-bound attention overlaps the PE-bound
            # projections of the next chunk.
            D = 6
            with tc.tile_pool(name="pss", bufs=2, space="PSUM") as pss, \
                 tc.tile_pool(name="pso", bufs=2, space="PSUM") as pso:
                p1 = tc.alloc_tile_pool(name="p1", bufs=1)
                xs = tc.alloc_tile_pool(name="xs", bufs=2)
                Wq_sb = p1.tile([128, NCT * 512], F32R, name="Wq_sb")
                Wk_sb = p1.tile([128, NCT * 512], F32R, name="Wk_sb")
                Wv_sb = p1.tile([128, NCT * 512], F32R, name="Wv_sb")
                for ct in range(NCT):
                    cs = slice(ct * 128, (ct + 1) * 128)
                    fs = slice(ct * 512, (ct + 1) * 512)
                    nc.sync.dma_start(out=Wq_sb[:, fs], in_=WqT_d[cs, :].bitcast(F32R))
                    nc.sync.dma_start(out=Wk_sb[:, fs], in_=WkT_d[cs, :].bitcast(F32R))
                    nc.sync.dma_start(out=Wv_sb[:, fs], in_=WvT_d[cs, :].bitcast(F32R))

                for ch in range(4):
                    t0 = ch * 512
                    xc = xs.tile([128, NCT * 512], F32R, name="xc", tag="xc")
                    for ct in range(NCT):
                        nc.sync.dma_start(
                            out=xc[:, ct * 512:(ct + 1) * 512],
                            in_=xT_d[ct * 128:(ct + 1) * 128, t0:t0 + 512].bitcast(F32R),
                        )
                    # Q^T / K^T for this chunk (bf16 out with scale+bias on DVE)
                    for g in range(4):
                        pq = pss.tile([128, 512], F32, name="pq", tag="sAB")
                        for ct in range(NCT):
                            nc.tensor.matmul(
                                out=pq[:],
                                lhsT=Wq_sb[:, ct * 512 + g * 128: ct * 512 + (g + 1) * 128],
                                rhs=xc[:, ct * 512:(ct + 1) * 512],
                                start=(ct == 0), stop=(ct == NCT - 1),
                            )
                        nc.vector.tensor_scalar(
                            out=QT[:, g * T + t0: g * T + t0 + 512], in0=pq[:],
                            scalar1=SCALE, scalar2=bq_sb[:, g:g + 1],
                            op0=ALU.mult, op1=ALU.add,
                        )
                        pk = pss.tile([128, 512], F32, name="pk", tag="sAB")
                        for ct in range(NCT):
                            nc.tensor.matmul(
                                out=pk[:],
                                lhsT=Wk_sb[:, ct * 512 + g * 128: ct * 512 + (g + 1) * 128],
                                rhs=xc[:, ct * 512:(ct + 1) * 512],
                                start=(ct == 0), stop=(ct == NCT - 1),
                            )
                        nc.vector.tensor_scalar(
                            out=KT[:, g * T + t0: g * T + t0 + 512], in0=pk[:],
                            scalar1=bk_sb[:, g:g + 1], scalar2=None, op0=ALU.add,
                        )
                    # V for this chunk (+ bias via K=1 matmul)
                    for ts in range(4):
                        kt = ch * 4 + ts
                        pv_ = pss.tile([128, 512], F32, name="pv_", tag="sAB")
                        for ct in range(NCT):
                            nc.tensor.matmul(
                                out=pv_[:],
                                lhsT=xc[:, ct * 512 + ts * 128: ct * 512 + ts * 128 + 128],
                                rhs=Wv_sb[:, ct * 512:(ct + 1) * 512],
                                start=(ct == 0), stop=False,
                            )
                        nc.tensor.matmul(
                            out=pv_[:], lhsT=ones128[:], rhs=bv_sb[:],
                            start=False, stop=True,
                        )
                        nc.vector.tensor_copy(Vr[:, kt, :, 0:64], pv_[:])

                xs.release()
                p1.release()
                es = tc.alloc_tile_pool(name="es", bufs=D + 1)
                rp = tc.alloc_tile_pool(name="rp", bufs=2)
                # ---- causal attention, descending q-chunks ----
                for qc in (3, 2, 1, 0):
                    q0 = qc * 512
                    kmax = 4 * qc + 4
                    for g in range(4):
                        gq = g * T
                        oAB = pso.tile([65, 1024], F32, name="oAB", tag="o")
                        e_l = [None] * kmax
                        off_l = [None] * kmax
                        for step in range(kmax + D):
                            if step < kmax:
                                kt = step
                                k0 = kt * 128
                                toff = 128 * (kt - 4 * qc) if kt >= 4 * qc else 0
                                off_l[kt] = toff
                                diag = kt >= 4 * qc
                                sAB = pss.tile([128, 1024], F32, name="sAB", tag="sAB")
                                nc.tensor.matmul(
                                    out=sAB[:, toff:512],
                                    lhsT=KT[0:64, gq + k0: gq + k0 + 128],
                                    rhs=QT[0:64, gq + q0 + toff: gq + q0 + 512],
                                    start=True, stop=True,
                                )
                                nc.tensor.matmul(
                                    out=sAB[:, 512 + toff:1024],
                                    lhsT=KT[64:128, gq + k0: gq + k0 + 128],
                                    rhs=QT[64:128, gq + q0 + toff: gq + q0 + 512],
                                    start=True, stop=True, tile_position=(64, 0),
                                )
                                if diag:
                                    # additive causal mask: only the 128-wide
                                    # diagonal band can contain masked entries
                                    nc.vector.tensor_add(
                                        sAB[:, toff:toff + 128],
                                        sAB[:, toff:toff + 128], maskf_sb[:])
                                    nc.vector.tensor_add(
                                        sAB[:, 512 + toff:512 + toff + 128],
                                        sAB[:, 512 + toff:512 + toff + 128], maskf_sb[:])
                                eAB = es.tile([128, 1024], F32R, name="eAB", tag="eAB")
                                s3 = sAB.rearrange("p (h w) -> p h w", h=2, w=512)
                                e3 = eAB.rearrange("p (h w) -> p h w", h=2, w=512)
                                nc.scalar.activation(
                                    e3[:, :, toff:512], s3[:, :, toff:512], AF.Exp,
                                    bias=pad_sb[:, kt:kt + 1])
                                e_l[kt] = eAB
                            pv = step - D
                            if 0 <= pv < kmax:
                                toff = off_l[pv]
                                vbase = pv * 520
                                nc.tensor.matmul(
                                    out=oAB[:, toff:512],
                                    lhsT=Vt[:, vbase + 130 * g: vbase + 130 * g + 65],
                                    rhs=e_l[pv][:, toff:512],
                                    start=(pv == 0), stop=(pv == kmax - 1),
                                )
                                nc.tensor.matmul(
                                    out=oAB[:, 512 + toff:1024],
                                    lhsT=Vt[:, vbase + 130 * g + 65: vbase + 130 * g + 130],
                                    rhs=e_l[pv][:, 512 + toff:1024],
                                    start=(pv == 0), stop=(pv == kmax - 1),
                                )
                        # epilogue: normalize by rowsum (row 64), write Y^T
                        rA = rp.tile([1, 512], F32, name="rA", tag="rA")
                        rB = rp.tile([1, 512], F32, name="rB", tag="rB")
                        nc.vector.reciprocal(rA[:], oAB[64:65, 0:512])
                        nc.vector.reciprocal(rB[:], oAB[64:65, 512:1024])
                        rbA = rp.tile([64, 512], F32, name="rbA", tag="rbA")
                        rbB = rp.tile([64, 512], F32, name="rbB", tag="rbB")
                        nc.gpsimd.partition_broadcast(rbA[:], rA[:])
                        nc.gpsimd.partition_broadcast(rbB[:], rB[:])
                        nc.vector.tensor_mul(
                            YT[0:64, gq + q0: gq + q0 + 512],
                            oAB[0:64, 0:512], rbA[:])
                        nc.vector.tensor_mul(
                            YT[64:128, gq + q0: gq + q0 + 512],
                            oAB[0:64, 512:1024], rbB[:])

                rp.release()
                es.release()

            # ---------------- output projection tail ----------------
            with tc.tile_pool(name="p3", bufs=1) as p3, \
                 tc.tile_pool(name="ob2", bufs=3) as obp2, \
                 tc.tile_pool(name="psp", bufs=4, space="PSUM") as psp:
                Wp_sb = p3.tile([128, 4 * C], F32R, name="Wp_sb")
                for g in range(4):
                    nc.sync.dma_start(
                        out=Wp_sb[:, g * C:(g + 1) * C],
                        in_=WpT_d[g * 128:(g + 1) * 128, :].bitcast(F32R),
                    )
                for tt in range(16):
                    for oc in range(2):
                        po = psp.tile([128, 512], F32, name="po", tag="po")
                        for g in range(4):
                            nc.tensor.matmul(
                                out=po[:],
                                lhsT=YT[:, g * T + tt * 128: g * T + tt * 128 + 128],
                                rhs=Wp_sb[:, g * C + oc * 512: g * C + oc * 512 + 512],
                                start=(g == 0), stop=(g == 3),
                            )
                        ob = obp2.tile([128, 512], F32, name="ob", tag="ob")
                        if (tt + oc) % 2 == 0:
                            nc.vector.tensor_copy(ob[:], po[:])
                        else:
                            nc.scalar.copy(ob[:], po[:])
                        nc.sync.dma_start(
                            out=out_d[tt * 128:(tt + 1) * 128,
                                      oc * 512:(oc + 1) * 512],
                            in_=ob[:],
                        )

    nc.compile()
    return nc


def _in_maps(x, Wk, bk, Wq, bq, Wv, bv, Wp, bp, padding_mask):
    maps = []
    mask_cols = np.arange(896)[None, :]
    mask_rows = np.arange(128)[:, None]
    maskneg = np.where(mask_rows <= mask_cols - 384, 0.0, NEG).astype(np.float32)
    for core in range(8):
        b, half = divmod(core, 2)
        hs = slice(half * IC, (half + 1) * IC)
        maps.append({
            "xT": np.ascontiguousarray(x[b].T),
            "WqT": np.ascontiguousarray(Wq[hs, :].T),
            "WkT": np.ascontiguousarray(Wk[hs, :].T),
            "WvT": np.ascontiguousarray(Wv[hs, :].T),
            "WpT": np.ascontiguousarray(Wp[:, hs].T),
            "bqs": np.ascontiguousarray((bq[hs] * SCALE).reshape(4, 128).T),
            "bks": np.ascontiguousarray(bk[hs].reshape(4, 128).T),
            "bvr": bv[hs].reshape(1, IC).copy(),
            "padb": np.ascontiguousarray(
                np.where(padding_mask[b] != 0, 0.0, NEG)
                .astype(np.float32).reshape(NKT, 128).T),
            "maskneg": maskneg,
            "ones128": np.ones((1, 128), np.float32),
            "ones8": np.ones((128, 8), np.float32),
            "ident": np.eye(128).astype(ml_dtypes.bfloat16),
            "maskb": maskneg[:, 384:512].astype(ml_dtypes.bfloat16),
        })
    return maps


def _run(inputs, trace=False, **kw):
    if "nc" not in _CACHE:
        _CACHE["nc"] = _build()
    nc = _CACHE["nc"]
    ins = {k: np.asarray(v, dtype=np.float32) if k != "padding_mask"
           else np.asarray(v) for k, v in inputs.items()}
    maps = _in_maps(**ins)
    res = run_bass_kernel_spmd(nc, maps, core_ids=list(range(8)), trace=trace, **kw)
    bp = np.asarray(inputs["bp"], np.float32)
    y = np.empty((B, T, C), np.float32)
    for b in range(B):
        y[b] = res.results[2 * b]["out"] + res.results[2 * b + 1]["out"] + bp
    return y, res


def kernel(**inputs):
    y, _ = _run(inputs, trace=False)
    return y


# revision 26
# speedup vs baseline: 1.1339x; 1.1339x over previous
"""Causal self-attention TRN2 Bass kernel.

Problem: B=4, T=2048, C=1024, H=16 heads (HD=64), torch-Linear semantics
(y = x @ W.T + b), causal + padding mask, softmax, output projection.

Sharding: 8 cores = (batch b in 0..3) x (head-half in 0..1). Each core
handles one batch and 8 heads (512 of the 1024 channels of QKV / of the
contraction dim of the output projection). The two half-cores of a batch
produce partial output projections that the host sums (plus bp).

Per-core kernel (all matmuls in float32r — full PE rate, ~1.5e-4 rel):
  Phase 1: QKV projections.
    Q^T, K^T computed head-major ([outch, T]) so attention needs no
    transposes; V computed token-major ([T, outch]) with an interleaved
    ones column per head (rowsum trick). Attention scale (1/8) and bias
    are folded in during the PSUM->SBUF copy on DVE; V bias is added via
    a K=1 ones-outer-product matmul into PSUM.
  Phase 2: flash-style causal attention per head-pair g (2 heads packed
    on PE rows 0-63 / 64-127 via tile_position for the K=64 S^T matmuls).
    S^T[k,q] = K Q^T tiles; causal masking adds -1e30 to PSUM on the
    diagonal tiles; exp on ACT (no max subtraction needed: |S|<~3);
    O_unnorm^T[d,q] plus rowsum row via [V | 1] stationary; normalization
    via reciprocal + K=1 broadcast matmul + DVE multiply into Y^T.
  Phase 3: output projection from Y^T tiles (stationary) vs Wp^T slices.
"""

import ml_dtypes
import numpy as np

import concourse.mybir as mybir
import concourse.tile as tile
from concourse import bacc
from concourse.bass_utils import run_bass_kernel_spmd

F32 = mybir.dt.float32
F32R = mybir.dt.float32r
BF16 = mybir.dt.bfloat16
AF = mybir.ActivationFunctionType
ALU = mybir.AluOpType

B, T, C, H = 4, 2048, 1024, 16
HD = C // H          # 64
IC = C // 2          # 512 channels per core (8 heads)
NKT = T // 128       # 16 k-tiles
NQC = T // 512       # 4 q-chunks
NCT = C // 128       # 8 contraction tiles for QKV
NEG = -1.0e30
SCALE = 1.0 / np.sqrt(HD)

_CACHE = {}


def _build():
    nc = bacc.Bacc("TRN2", target_bir_lowering=False, debug=False)

    xT_d = nc.dram_tensor("xT", [C, T], F32, kind="ExternalInput").ap()
    WqT_d = nc.dram_tensor("WqT", [C, IC], F32, kind="ExternalInput").ap()
    WkT_d = nc.dram_tensor("WkT", [C, IC], F32, kind="ExternalInput").ap()
    WvT_d = nc.dram_tensor("WvT", [C, IC], F32, kind="ExternalInput").ap()
    WpT_d = nc.dram_tensor("WpT", [IC, C], F32, kind="ExternalInput").ap()
    bq_d = nc.dram_tensor("bqs", [128, 4], F32, kind="ExternalInput").ap()
    bk_d = nc.dram_tensor("bks", [128, 4], F32, kind="ExternalInput").ap()
    bv_d = nc.dram_tensor("bvr", [1, IC], F32, kind="ExternalInput").ap()
    pad_d = nc.dram_tensor("padb", [128, NKT], F32, kind="ExternalInput").ap()
    mask_d = nc.dram_tensor("maskneg", [128, 896], F32, kind="ExternalInput").ap()
    ones128_d = nc.dram_tensor("ones128", [1, 128], F32, kind="ExternalInput").ap()
    ones8f_d = nc.dram_tensor("ones8", [128, 8], F32, kind="ExternalInput").ap()
    ident_d = nc.dram_tensor("ident", [128, 128], BF16, kind="ExternalInput").ap()
    maskb_d = nc.dram_tensor("maskb", [128, 128], BF16, kind="ExternalInput").ap()
    out_d = nc.dram_tensor("out", [T, C], F32, kind="ExternalOutput").ap()

    with tile.TileContext(nc) as tc:
        with tc.tile_pool(name="pp", bufs=1) as pp:
            # Persistent SBUF state
            QT = pp.tile([128, 4 * T], BF16, name="QT")     # 4 head-pair tiles
            KT = pp.tile([128, 4 * T], BF16, name="KT")
            Vt = pp.tile([128, NKT * 520], F32R, name="Vt")  # [V|1] x 8 heads
            YT = pp.tile([128, 4 * T], F32R, name="YT")
            bq_sb = pp.tile([128, 4], F32, name="bq_sb")
            bk_sb = pp.tile([128, 4], F32, name="bk_sb")
            bv_sb = pp.tile([1, IC], F32R, name="bv_sb")
            pad_sb = pp.tile([128, NKT], F32, name="pad_sb")
            ones128 = pp.tile([1, 128], F32R, name="ones128")
            mask_sb = pp.tile([128, 128], BF16, name="mask_sb")
            ident_sb = pp.tile([128, 128], BF16, name="ident_sb")
            nc.sync.dma_start(out=bq_sb[:], in_=bq_d)
            nc.sync.dma_start(out=bk_sb[:], in_=bk_d)
            nc.sync.dma_start(out=bv_sb[:], in_=bv_d.bitcast(F32R))
            nc.sync.dma_start(out=pad_sb[:], in_=pad_d)
            nc.sync.dma_start(out=ones128[:], in_=ones128_d.bitcast(F32R))
            nc.sync.dma_start(out=mask_sb[:], in_=maskb_d)
            nc.sync.dma_start(out=ident_sb[:], in_=ident_d)
            Vr = Vt.rearrange("p (k h c) -> p k h c", k=NKT, h=8, c=65)
            for kt in range(NKT):
                nc.sync.dma_start(out=Vr[:, kt, :, 64], in_=ones8f_d.bitcast(F32R))

            # ---- interleaved QKV production + causal attention ----
            # For each T-chunk ch: project Q^T/K^T/V for those 512 tokens,
            # then immediately run attention q-chunk qc=ch (its K range only
            # needs chunks <= ch). ACT-bound attention overlaps the PE-bound
            # projections of the next chunk.
            D = 6
            with tc.tile_pool(name="pss", bufs=2, space="PSUM") as pss, \
                 tc.tile_pool(name="pso", bufs=2, space="PSUM") as pso:
                p1 = tc.alloc_tile_pool(name="p1", bufs=1)
                xs = tc.alloc_tile_pool(name="xs", bufs=2)
                Wq_sb = p1.tile([128, NCT * 512], F32R, name="Wq_sb")
                Wk_sb = p1.tile([128, NCT * 512], F32R, name="Wk_sb")
                Wv_sb = p1.tile([128, NCT * 512], F32R, name="Wv_sb")
                for ct in range(NCT):
                    cs = slice(ct * 128, (ct + 1) * 128)
                    fs = slice(ct * 512, (ct + 1) * 512)
                    nc.sync.dma_start(out=Wq_sb[:, fs], in_=WqT_d[cs, :].bitcast(F32R))
                    nc.sync.dma_start(out=Wk_sb[:, fs], in_=WkT_d[cs, :].bitcast(F32R))
                    nc.sync.dma_start(out=Wv_sb[:, fs], in_=WvT_d[cs, :].bitcast(F32R))

                for ch in range(4):
                    t0 = ch * 512
                    xc = xs.tile([128, NCT * 512], F32R, name="xc", tag="xc")
                    for ct in range(NCT):
                        nc.sync.dma_start(
                            out=xc[:, ct * 512:(ct + 1) * 512],
                            in_=xT_d[ct * 128:(ct + 1) * 128, t0:t0 + 512].bitcast(F32R),
                        )
                    # Q^T / K^T for this chunk (bf16 out with scale+bias on DVE)
                    for g in range(4):
                        pq = pss.tile([128, 512], F32, name="pq", tag="sAB")
                        for ct in range(NCT):
                            nc.tensor.matmul(
                                out=pq[:],
                                lhsT=Wq_sb[:, ct * 512 + g * 128: ct * 512 + (g + 1) * 128],
                                rhs=xc[:, ct * 512:(ct + 1) * 512],
                                start=(ct == 0), stop=(ct == NCT - 1),
                            )
                        nc.vector.tensor_scalar(
                            out=QT[:, g * T + t0: g * T + t0 + 512], in0=pq[:],
                            scalar1=SCALE, scalar2=bq_sb[:, g:g + 1],
                            op0=ALU.mult, op1=ALU.add,
                        )
                        pk = pss.tile([128, 512], F32, name="pk", tag="sAB")
                        for ct in range(NCT):
                            nc.tensor.matmul(
                                out=pk[:],
                                lhsT=Wk_sb[:, ct * 512 + g * 128: ct * 512 + (g + 1) * 128],
                                rhs=xc[:, ct * 512:(ct + 1) * 512],
                                start=(ct == 0), stop=(ct == NCT - 1),
                            )
                        nc.vector.tensor_scalar(
                            out=KT[:, g * T + t0: g * T + t0 + 512], in0=pk[:],
                            scalar1=bk_sb[:, g:g + 1], scalar2=None, op0=ALU.add,
                        )
                    # V for this chunk (+ bias via K=1 matmul)
                    for ts in range(4):
                        kt = ch * 4 + ts
                        pv_ = pss.tile([128, 512], F32, name="pv_", tag="sAB")
                        for ct in range(NCT):
                            nc.tensor.matmul(
                                out=pv_[:],
                                lhsT=xc[:, ct * 512 + ts * 128: ct * 512 + ts * 128 + 128],
                                rhs=Wv_sb[:, ct * 512:(ct + 1) * 512],
                                start=(ct == 0), stop=False,
                            )
                        nc.tensor.matmul(
                            out=pv_[:], lhsT=ones128[:], rhs=bv_sb[:],
                            start=False, stop=True,
                        )
                        nc.vector.tensor_copy(Vr[:, kt, :, 0:64], pv_[:])

                xs.release()
                p1.release()
                es = tc.alloc_tile_pool(name="es", bufs=D + 1)
                rp = tc.alloc_tile_pool(name="rp", bufs=2)
                # ---- causal attention, descending q-chunks ----
                for qc in (3, 2, 1, 0):
                    q0 = qc * 512
                    kmax = 4 * qc + 4
                    for g in range(4):
                        gq = g * T
                        oAB = pso.tile([65, 1024], F32, name="oAB", tag="o")
                        e_l = [None] * kmax
                        off_l = [None] * kmax
                        for step in range(kmax + D):
                            if step < kmax:
                                kt = step
                                k0 = kt * 128
                                toff = 128 * (kt - 4 * qc) if kt >= 4 * qc else 0
                                off_l[kt] = toff
                                diag = kt >= 4 * qc
                                sAB = pss.tile([128, 1024], F32, name="sAB", tag="sAB")
                                nc.tensor.matmul(
                                    out=sAB[:, toff:512],
                                    lhsT=KT[0:64, gq + k0: gq + k0 + 128],
                                    rhs=QT[0:64, gq + q0 + toff: gq + q0 + 512],
                                    start=True, stop=not diag,
                                )
                                nc.tensor.matmul(
                                    out=sAB[:, 512 + toff:1024],
                                    lhsT=KT[64:128, gq + k0: gq + k0 + 128],
                                    rhs=QT[64:128, gq + q0 + toff: gq + q0 + 512],
                                    start=True, stop=not diag, tile_position=(64, 0),
                                )
                                if diag:
                                    # additive causal mask on the 128-wide
                                    # diagonal band, via identity matmul
                                    nc.tensor.matmul(
                                        out=sAB[:, toff:toff + 128],
                                        lhsT=ident_sb[:], rhs=mask_sb[:],
                                        start=False, stop=True,
                                    )
                                    nc.tensor.matmul(
                                        out=sAB[:, 512 + toff:512 + toff + 128],
                                        lhsT=ident_sb[:], rhs=mask_sb[:],
                                        start=False, stop=True,
                                    )
                                eAB = es.tile([128, 1024], F32R, name="eAB", tag="eAB")
                                s3 = sAB.rearrange("p (h w) -> p h w", h=2, w=512)
                                e3 = eAB.rearrange("p (h w) -> p h w", h=2, w=512)
                                nc.scalar.activation(
                                    e3[:, :, toff:512], s3[:, :, toff:512], AF.Exp,
                                    bias=pad_sb[:, kt:kt + 1])
                                e_l[kt] = eAB
                            pv = step - D
                            if 0 <= pv < kmax:
                                toff = off_l[pv]
                                vbase = pv * 520
                                nc.tensor.matmul(
                                    out=oAB[:, toff:512],
                                    lhsT=Vt[:, vbase + 130 * g: vbase + 130 * g + 65],
                                    rhs=e_l[pv][:, toff:512],
                                    start=(pv == 0), stop=(pv == kmax - 1),
                                )
                                nc.tensor.matmul(
                                    out=oAB[:, 512 + toff:1024],
                                    lhsT=Vt[:, vbase + 130 * g + 65: vbase + 130 * g + 130],
                                    rhs=e_l[pv][:, 512 + toff:1024],
                                    start=(pv == 0), stop=(pv == kmax - 1),
                                )
                        # epilogue: normalize by rowsum (row 64), write Y^T
                        rA = rp.tile([1, 512], F32, name="rA", tag="rA")
                        rB = rp.tile([1, 512], F32, name="rB", tag="rB")
                        nc.vector.reciprocal(rA[:], oAB[64:65, 0:512])
                        nc.vector.reciprocal(rB[:], oAB[64:65, 512:1024])
                        rbA = rp.tile([64, 512], F32, name="rbA", tag="rbA")
                        rbB = rp.tile([64, 512], F32, name="rbB", tag="rbB")
                        nc.gpsimd.partition_broadcast(rbA[:], rA[:])
                        nc.gpsimd.partition_broadcast(rbB[:], rB[:])
                        nc.vector.tensor_mul(
                            YT[0:64, gq + q0: gq + q0 + 512],
                            oAB[0:64, 0:512], rbA[:])
                        nc.vector.tensor_mul(
                            YT[64:128, gq + q0: gq + q0 + 512],
                            oAB[0:64, 512:1024], rbB[:])

                rp.release()
                es.release()

            # ---------------- output projection tail ----------------
            with tc.tile_pool(name="p3", bufs=1) as p3, \
                 tc.tile_pool(name="ob2", bufs=3) as obp2, \
                 tc.tile_pool(name="psp", bufs=4, space="PSUM") as psp:
                Wp_sb = p3.tile([128, 4 * C], F32R, name="Wp_sb")
                for g in range(4):
                    nc.sync.dma_start(
                        out=Wp_sb[:, g * C:(g + 1) * C],
                        in_=WpT_d[g * 128:(g + 1) * 128, :].bitcast(F32R),
                    )
                for tt in range(16):
                    for oc in range(2):
                        po = psp.tile([128, 512], F32, name="po", tag="po")
                        for g in range(4):
                            nc.tensor.matmul(
                                out=po[:],
                                lhsT=YT[:, g * T + tt * 128: g * T + tt * 128 + 128],
                                rhs=Wp_sb[:, g * C + oc * 512: g * C + oc * 512 + 512],
                                start=(g == 0), stop=(g == 3),
                            )
                        ob = obp2.tile([128, 512], F32, name="ob", tag="ob")
                        if (tt + oc) % 2 == 0:
                            nc.vector.tensor_copy(ob[:], po[:])
                        else:
                            nc.scalar.copy(ob[:], po[:])
                        nc.sync.dma_start(
                            out=out_d[tt * 128:(tt + 1) * 128,
                                      oc * 512:(oc + 1) * 512],
                            in_=ob[:],
                        )

    nc.compile()
    return nc


def _in_maps(x, Wk, bk, Wq, bq, Wv, bv, Wp, bp, padding_mask):
    maps = []
    mask_cols = np.arange(896)[None, :]
    mask_rows = np.arange(128)[:, None]
    maskneg = np.where(mask_rows <= mask_cols - 384, 0.0, NEG).astype(np.float32)
    for core in range(8):
        b, half = divmod(core, 2)
        hs = slice(half * IC, (half + 1) * IC)
        maps.append({
            "xT": np.ascontiguousarray(x[b].T),
            "WqT": np.ascontiguousarray(Wq[hs, :].T),
            "WkT": np.ascontiguousarray(Wk[hs, :].T),
            "WvT": np.ascontiguousarray(Wv[hs, :].T),
            "WpT": np.ascontiguousarray(Wp[:, hs].T),
            "bqs": np.ascontiguousarray((bq[hs] * SCALE).reshape(4, 128).T),
            "bks": np.ascontiguousarray(bk[hs].reshape(4, 128).T),
            "bvr": bv[hs].reshape(1, IC).copy(),
            "padb": np.ascontiguousarray(
                np.where(padding_mask[b] != 0, 0.0, NEG)
                .astype(np.float32).reshape(NKT, 128).T),
            "maskneg": maskneg,
            "ones128": np.ones((1, 128), np.float32),
            "ones8": np.ones((128, 8), np.float32),
            "ident": np.eye(128).astype(ml_dtypes.bfloat16),
            "maskb": maskneg[:, 384:512].astype(ml_dtypes.bfloat16),
        })
    return maps


def _run(inputs, trace=False, **kw):
    if "nc" not in _CACHE:
        _CACHE["nc"] = _build()
    nc = _CACHE["nc"]
    ins = {k: np.asarray(v, dtype=np.float32) if k != "padding_mask"
           else np.asarray(v) for k, v in inputs.items()}
    maps = _in_maps(**ins)
    res = run_bass_kernel_spmd(nc, maps, core_ids=list(range(8)), trace=trace, **kw)
    bp = np.asarray(inputs["bp"], np.float32)
    y = np.empty((B, T, C), np.float32)
    for b in range(B):
        y[b] = res.results[2 * b]["out"] + res.results[2 * b + 1]["out"] + bp
    return y, res


def kernel(**inputs):
    y, _ = _run(inputs, trace=False)
    return y
